# revision 1
# baseline (speedup 1.0000x reference)
"""Adaptive-softmax CE loss on 8 TRN2 NeuronCores.

Strategy: data-parallel over tokens (4096 tokens -> 512/core), weights
replicated, no collectives.  Per core the Bass/Tile kernel computes
  - hT0 = tail0_w1 @ w_in_shard.T, hT1 = tail1_w1 @ w_in_shard.T  (bf16)
  - label logits via elementwise-mul + ones-vector matmul (partition reduce)
  - streamed vocab-chunk logit matmuls for head/tail0/tail1 with fused
    exp + per-token row-sum on the ScalarEngine (activation accum_out)
and exports per-token sum-exp partials + label dots.  The host finishes
with log() in float64, applies the cluster masks, and averages.

Numerics: logits computed in bf16 (fp32 PSUM accumulate).  Host-side
emulation gives |rel err| ~5e-7 vs the fp32 reference.  max|logit| < 6,
so no max-subtraction is needed inside logsumexp.
"""

import numpy as np
import ml_dtypes

CUTOFF = [2000, 10000, 50000]
N_TOK = 4096
D = 1024
N_CORES = 8
TOK_PER_CORE = N_TOK // N_CORES          # 512
N_BLK = TOK_PER_CORE // 128              # 4 token blocks of 128
KX = 9                                   # k-chunks of augmented input (1152/128)
K0 = 8                                   # k-chunks of tail0 proj (1024/128)
K1 = 2                                   # k-chunks of tail1 proj (256/128)
SUP = 2048                               # ACT super-chunk width
HEAD_PAD, T0_PAD, T1_PAD = 2048, 8192, 40960
N_SUP0 = T0_PAD // SUP                   # 4
N_SUP1 = T1_PAD // SUP                   # 20
COLS_PER_BLK = 1 + N_SUP0 + N_SUP1       # 25
S_COLS = N_BLK * COLS_PER_BLK            # 100

BF16 = ml_dtypes.bfloat16

_cache = {}


def _build_nc():
    import concourse.bacc as bacc
    import concourse.mybir as mybir
    from concourse import tile

    dt = mybir.dt
    nc = bacc.Bacc(None)

    xt_p = nc.declare_dram_parameter("xt", [KX, 128, TOK_PER_CORE], dt.bfloat16, isOutput=False)
    w1t0_p = nc.declare_dram_parameter("w1t0", [K0, 128, 1024], dt.bfloat16, isOutput=False)
    w1t1_p = nc.declare_dram_parameter("w1t1", [K0, 128, 256], dt.bfloat16, isOutput=False)
    w2t0_p = nc.declare_dram_parameter("w2t0", [K0, 128, T0_PAD], dt.bfloat16, isOutput=False)
    w2t1_p = nc.declare_dram_parameter("w2t1", [K1, 128, T1_PAD], dt.bfloat16, isOutput=False)
    hwt_p = nc.declare_dram_parameter("hwt", [KX, 128, HEAD_PAD], dt.bfloat16, isOutput=False)
    gall_p = nc.declare_dram_parameter("gall", [KX + K0 + K1, 128, TOK_PER_CORE], dt.bfloat16, isOutput=False)
    out_s_p = nc.declare_dram_parameter("out_s", [128, S_COLS], dt.float32, isOutput=True)
    out_ll_p = nc.declare_dram_parameter("out_ll", [1, TOK_PER_CORE], dt.float32, isOutput=True)

    EXP = mybir.ActivationFunctionType.Exp
    MULT = mybir.AluOpType.mult
    import concourse.bass as bass
    PSUM = bass.MemorySpace.PSUM

    with tile.TileContext(nc) as tc:
        with (
            tc.tile_pool(name="res", bufs=1) as res,
            tc.tile_pool(name="w2s0", bufs=2) as w2s0,
            tc.tile_pool(name="w2s1", bufs=3) as w2s1,
            tc.tile_pool(name="gs", bufs=4) as gs,
            tc.tile_pool(name="prs", bufs=3) as prs,
        ):
            # ---- resident SBUF tensors ----
            xt = res.tile([128, KX, TOK_PER_CORE], dt.bfloat16, tag="xt")
            w1t0 = res.tile([128, K0, 1024], dt.bfloat16, tag="w1t0")
            w1t1 = res.tile([128, K0, 256], dt.bfloat16, tag="w1t1")
            hwt = res.tile([128, KX, HEAD_PAD], dt.bfloat16, tag="hwt")
            ht0 = res.tile([128, K0, TOK_PER_CORE], dt.bfloat16, tag="ht0")
            ht1 = res.tile([128, K1, TOK_PER_CORE], dt.bfloat16, tag="ht1")
            sall = res.tile([128, S_COLS], dt.float32, tag="sall")
            ll = res.tile([1, TOK_PER_CORE], dt.float32, tag="ll")
            ones = res.tile([128, 1], dt.bfloat16, tag="ones")

            nc.gpsimd.memset(ones[:], 1.0)
            for k in range(KX):
                nc.sync.dma_start(xt[:, k, :], xt_p[k])
            for k in range(K0):
                nc.sync.dma_start(w1t0[:, k, :], w1t0_p[k])
                nc.sync.dma_start(w1t1[:, k, :], w1t1_p[k])
            for k in range(KX):
                nc.sync.dma_start(hwt[:, k, :], hwt_p[k])

            # ---- phase A: hT = w1 @ x.T (transposed activations) ----
            with tc.tile_pool(name="pa", bufs=4, space=PSUM) as pa:
                for w1t, kk, ht in ((w1t0, K0, ht0), (w1t1, K1, ht1)):
                    for m in range(kk):
                        pt = pa.tile([128, TOK_PER_CORE], dt.float32, tag="pa")
                        for k in range(K0):
                            nc.tensor.matmul(
                                pt[:],
                                lhsT=w1t[:, k, m * 128:(m + 1) * 128],
                                rhs=xt[:, k, :],
                                start=(k == 0),
                                stop=(k == K0 - 1),
                            )
                        nc.vector.tensor_copy(ht[:, m, :], pt[:])

            # ---- phase B: label-logit dots via ones-matmul ----
            lhs_chunks = (
                [xt[:, k, :] for k in range(KX)]
                + [ht0[:, k, :] for k in range(K0)]
                + [ht1[:, k, :] for k in range(K1)]
            )
            with tc.tile_pool(name="pb", bufs=1, space=PSUM) as pb:
                pll = pb.tile([1, TOK_PER_CORE], dt.float32, tag="pb")
                n = len(lhs_chunks)
                for i, lhs in enumerate(lhs_chunks):
                    gt = gs.tile([128, TOK_PER_CORE], dt.bfloat16, tag="g")
                    nc.sync.dma_start(gt[:], gall_p[i])
                    pr = prs.tile([128, TOK_PER_CORE], dt.bfloat16, tag="pr")
                    nc.vector.tensor_tensor(pr[:], lhs, gt[:], op=MULT)
                    nc.tensor.matmul(
                        pll[:], lhsT=ones[:], rhs=pr[:],
                        start=(i == 0), stop=(i == n - 1),
                    )
                nc.vector.tensor_copy(ll[:], pll[:])

            # ---- phase C: streamed logits + fused exp/row-sum ----
            def logit_group(pc, width, b, kk, lhs3, rhs3, col):
                """matmul-accumulate [128,width] logits then exp+rowsum."""
                for sub in range(width // 512):
                    sl = slice(sub * 512, (sub + 1) * 512)
                    for k in range(kk):
                        nc.tensor.matmul(
                            pc[:, sl],
                            lhsT=lhs3[:, k, b * 128:(b + 1) * 128],
                            rhs=rhs3[:, k, sl],
                            start=(k == 0),
                            stop=(k == kk - 1),
                        )
                nc.scalar.activation(
                    pc[:, :width], pc[:, :width], EXP,
                    accum_out=sall[:, col:col + 1],
                )

            with tc.tile_pool(name="pc", bufs=2, space=PSUM) as pcp:
                # head: resident weights, augmented k (bias row)
                for b in range(N_BLK):
                    pc = pcp.tile([128, SUP], dt.float32, tag="pc")
                    logit_group(pc, SUP, b, KX, xt, hwt, b * COLS_PER_BLK)

                # tail0
                for sup in range(N_SUP0):
                    wt = w2s0.tile([128, K0, SUP], dt.bfloat16, tag="w0")
                    for k in range(K0):
                        nc.sync.dma_start(wt[:, k, :], w2t0_p[k, :, sup * SUP:(sup + 1) * SUP])
                    for b in range(N_BLK):
                        pc = pcp.tile([128, SUP], dt.float32, tag="pc")
                        logit_group(pc, SUP, b, K0, ht0, wt, b * COLS_PER_BLK + 1 + sup)

                # tail1
                for sup in range(N_SUP1):
                    wt = w2s1.tile([128, K1, SUP], dt.bfloat16, tag="w1")
                    for k in range(K1):
                        nc.sync.dma_start(wt[:, k, :], w2t1_p[k, :, sup * SUP:(sup + 1) * SUP])
                    for b in range(N_BLK):
                        pc = pcp.tile([128, SUP], dt.float32, tag="pc")
                        logit_group(pc, SUP, b, K1, ht1, wt, b * COLS_PER_BLK + 1 + N_SUP0 + sup)

            nc.sync.dma_start(out_s_p[:], sall[:])
            nc.sync.dma_start(out_ll_p[:], ll[:])

    nc.compile()
    return nc


def _prep_inputs(w_in, target, head_w, head_b, tail0_w1, tail0_w2, tail1_w1, tail1_w2):
    """Host-side shard + transpose + pad + bf16 cast. Returns in_maps."""
    f32 = np.float32
    w_in = np.asarray(w_in, f32)
    target = np.asarray(target).astype(np.int64)
    head_w = np.asarray(head_w, f32)
    head_b = np.asarray(head_b, f32)
    t0w1 = np.asarray(tail0_w1, f32)
    t0w2 = np.asarray(tail0_w2, f32)
    t1w1 = np.asarray(tail1_w1, f32)
    t1w2 = np.asarray(tail1_w2, f32)

    c0, c1, c2 = CUTOFF
    mask0 = (target >= c0) & (target < c1)
    mask1 = (target >= c1) & (target < c2)
    label0 = np.clip(target - c0, 0, c1 - c0 - 1)
    label1 = np.clip(target - c1, 0, c2 - c1 - 1)
    first_t = np.where(mask0, c0, np.where(mask1, c0 + 1, target))

    # label-gathered rows, masks folded in
    g0 = t0w2[label0] * mask0[:, None].astype(f32)     # [N_TOK, 1024]
    g1 = t1w2[label1] * mask1[:, None].astype(f32)     # [N_TOK, 256]
    gh = head_w[first_t]                               # [N_TOK, 1024]
    bh = head_b[first_t]                               # [N_TOK]

    # shared (replicated) weight layouts
    def chunks(a, k):      # [K*128, F] -> [K, 128, F]
        return np.ascontiguousarray(a.reshape(k, 128, a.shape[1])).astype(BF16)

    w1t0 = chunks(t0w1.T, K0)                          # [8,128,1024]
    w1t1 = chunks(t1w1.T, K0)                          # [8,128,256]
    w2t0 = np.zeros((1024, T0_PAD), f32)
    w2t0[:, :c1 - c0] = t0w2.T
    w2t0 = chunks(w2t0, K0)                            # [8,128,8192]
    w2t1 = np.zeros((256, T1_PAD), f32)
    w2t1[:, :c2 - c1] = t1w2.T
    w2t1 = chunks(w2t1, K1)                            # [2,128,40960]
    hwt = np.zeros((KX * 128, HEAD_PAD), f32)
    n_head = head_w.shape[0]
    hwt[:D, :n_head] = head_w.T
    hwt[D, :n_head] = head_b                           # bias row pairs with ones-row of xt
    hwt = chunks(hwt, KX)                              # [9,128,2048]

    in_maps = []
    for c in range(N_CORES):
        sl = slice(c * TOK_PER_CORE, (c + 1) * TOK_PER_CORE)
        xt = np.zeros((KX * 128, TOK_PER_CORE), f32)
        xt[:D] = w_in[sl].T
        xt[D] = 1.0                                    # augmented ones-row (bias)
        ght = np.zeros((KX * 128, TOK_PER_CORE), f32)
        ght[:D] = gh[sl].T
        ght[D] = bh[sl]
        gall = np.concatenate(
            [chunks(ght, KX), chunks(g0[sl].T, K0), chunks(g1[sl].T, K1)], axis=0
        )
        in_maps.append({
            "xt": chunks(xt, KX),
            "w1t0": w1t0, "w1t1": w1t1, "w2t0": w2t0, "w2t1": w2t1, "hwt": hwt,
            "gall": gall,
        })
    return in_maps, mask0, mask1


def _combine(results, mask0, mask1):
    """Host-side unshard: log in f64, apply masks, mean."""
    total = 0.0
    pad_h = HEAD_PAD - (CUTOFF[0] + 2)
    pad_0 = T0_PAD - (CUTOFF[1] - CUTOFF[0])
    pad_1 = T1_PAD - (CUTOFF[2] - CUTOFF[1])
    for c in range(N_CORES):
        S = results[c]["out_s"].astype(np.float64).reshape(128, N_BLK, COLS_PER_BLK)
        llv = results[c]["out_ll"].astype(np.float64).reshape(N_BLK, 128)
        Sh = S[:, :, 0] - pad_h                        # [128, N_BLK]
        S0 = S[:, :, 1:1 + N_SUP0].sum(-1) - pad_0
        S1 = S[:, :, 1 + N_SUP0:].sum(-1) - pad_1
        # token (p, b) -> global index c*512 + b*128 + p
        idx = (c * TOK_PER_CORE + np.arange(N_BLK)[None, :] * 128
               + np.arange(128)[:, None])
        m0 = mask0[idx]
        m1 = mask1[idx]
        nll = np.log(Sh) + m0 * np.log(S0) + m1 * np.log(S1) - llv.T
        total += nll.sum()
    return np.float32(total / N_TOK)


def _run(inputs, trace=False):
    from concourse.bass_utils import run_bass_kernel_spmd

    if "nc" not in _cache:
        _cache["nc"] = _build_nc()
    nc = _cache["nc"]
    in_maps, mask0, mask1 = _prep_inputs(**inputs)
    res = run_bass_kernel_spmd(nc, in_maps, core_ids=list(range(N_CORES)), trace=trace)
    loss = _combine(res.results, mask0, mask1)
    return loss, res


def kernel(**inputs) -> np.ndarray:
    loss, _ = _run(inputs, trace=False)
    return loss


# revision 2
# speedup vs baseline: 1.2105x; 1.2105x over previous
"""Adaptive-softmax CE loss on 8 TRN2 NeuronCores.

Strategy: data-parallel over tokens (4096 tokens -> 512/core), weights
replicated, no collectives.  Per core the Bass/Tile kernel computes
  - hT0 = tail0_w1 @ w_in_shard.T, hT1 = tail1_w1 @ w_in_shard.T  (bf16)
  - label logits via elementwise-mul + ones-vector matmul (partition reduce)
  - streamed vocab-chunk logit matmuls (fp8 DoubleRow, 2x PE rate, weights
    pre-scaled x64 on host; un-scaled for free via exp(x/64) on ScalarE)
    with fused exp + per-token row-sum (activation accum_out)
and exports per-token sum-exp partials + label dots.  The host finishes
with log() in float64, applies the cluster masks, and averages.

Numerics: fp8 logit noise ~0.04 abs gives a logsumexp convexity bias of
~1e-4 absolute on a loss of ~18 (~5e-6 rel); label dots stay bf16.
max|logit| < 6 so no max-subtraction is needed inside logsumexp.
"""

import numpy as np
import ml_dtypes

CUTOFF = [2000, 10000, 50000]
N_TOK = 4096
D = 1024
N_CORES = 8
TOK_PER_CORE = N_TOK // N_CORES          # 512
N_BLK = TOK_PER_CORE // 128              # 4 token blocks of 128
KX = 9                                   # k-chunks of augmented input (1152/128)
K0 = 8                                   # k-chunks of tail0 proj (1024/128)
K1 = 2                                   # k-chunks of tail1 proj (256/128)
SUP = 2048                               # ACT super-chunk width
HEAD_PAD, T0_PAD, T1_PAD = 2048, 8192, 40960
N_SUP0 = T0_PAD // SUP                   # 4
N_SUP1 = T1_PAD // SUP                   # 20
COLS_PER_BLK = 1 + N_SUP0 + N_SUP1       # 25
S_COLS = N_BLK * COLS_PER_BLK            # 100
WSCALE = 64.0                            # fp8 weight pre-scale (undone in exp)

BF16 = ml_dtypes.bfloat16
FP8 = ml_dtypes.float8_e4m3

_cache = {}


def _build_nc():
    import concourse.bass as bass
    import concourse.bacc as bacc
    import concourse.mybir as mybir
    from concourse import tile

    dt = mybir.dt
    nc = bacc.Bacc(None)

    xt_p = nc.declare_dram_parameter("xt", [KX, 128, TOK_PER_CORE], dt.bfloat16, isOutput=False)
    xt8_p = nc.declare_dram_parameter("xt8", [K0, 128, TOK_PER_CORE], dt.float8e4, isOutput=False)
    w1t0_p = nc.declare_dram_parameter("w1t0", [K0, 128, 1024], dt.bfloat16, isOutput=False)
    w1t1_p = nc.declare_dram_parameter("w1t1", [K0, 128, 256], dt.bfloat16, isOutput=False)
    w2t0_p = nc.declare_dram_parameter("w2t0", [K0, 128, T0_PAD], dt.float8e4, isOutput=False)
    w2t1_p = nc.declare_dram_parameter("w2t1", [K1, 128, T1_PAD], dt.float8e4, isOutput=False)
    hwt8_p = nc.declare_dram_parameter("hwt8", [K0, 128, HEAD_PAD], dt.float8e4, isOutput=False)
    hbias_p = nc.declare_dram_parameter("hbias", [128, HEAD_PAD], dt.bfloat16, isOutput=False)
    gall_p = nc.declare_dram_parameter("gall", [KX + K0 + K1, 128, TOK_PER_CORE], dt.bfloat16, isOutput=False)
    out_s_p = nc.declare_dram_parameter("out_s", [128, S_COLS], dt.float32, isOutput=True)
    out_ll_p = nc.declare_dram_parameter("out_ll", [1, TOK_PER_CORE], dt.float32, isOutput=True)

    EXP = mybir.ActivationFunctionType.Exp
    MULT = mybir.AluOpType.mult
    DR = mybir.MatmulPerfMode.DoubleRow
    PSUM = bass.MemorySpace.PSUM

    with tile.TileContext(nc) as tc:
        with (
            tc.tile_pool(name="res", bufs=1) as res,
            tc.tile_pool(name="w2s0", bufs=3) as w2s0,
            tc.tile_pool(name="w2s1", bufs=4) as w2s1,
            tc.tile_pool(name="gs", bufs=4) as gs,
            tc.tile_pool(name="prs", bufs=3) as prs,
        ):
            # ---- resident SBUF tensors ----
            xt = res.tile([128, KX, TOK_PER_CORE], dt.bfloat16, tag="xt")
            xt8 = res.tile([128, K0, TOK_PER_CORE], dt.float8e4, tag="xt8")
            w1t0 = res.tile([128, K0, 1024], dt.bfloat16, tag="w1t0")
            w1t1 = res.tile([128, K0, 256], dt.bfloat16, tag="w1t1")
            hwt8 = res.tile([128, K0, HEAD_PAD], dt.float8e4, tag="hwt8")
            hbias = res.tile([128, HEAD_PAD], dt.bfloat16, tag="hbias")
            ht0 = res.tile([128, K0, TOK_PER_CORE], dt.bfloat16, tag="ht0")
            ht1 = res.tile([128, K1, TOK_PER_CORE], dt.bfloat16, tag="ht1")
            ht0_8 = res.tile([128, K0, TOK_PER_CORE], dt.float8e4, tag="ht0_8")
            ht1_8 = res.tile([128, K1, TOK_PER_CORE], dt.float8e4, tag="ht1_8")
            sall = res.tile([128, S_COLS], dt.float32, tag="sall")
            ll = res.tile([1, TOK_PER_CORE], dt.float32, tag="ll")
            ones = res.tile([128, 1], dt.bfloat16, tag="ones")

            nc.gpsimd.memset(ones[:], 1.0)
            for k in range(KX):
                nc.sync.dma_start(xt[:, k, :], xt_p[k])
            for k in range(K0):
                nc.sync.dma_start(xt8[:, k, :], xt8_p[k])
                nc.sync.dma_start(w1t0[:, k, :], w1t0_p[k])
                nc.sync.dma_start(w1t1[:, k, :], w1t1_p[k])
                nc.sync.dma_start(hwt8[:, k, :], hwt8_p[k])
            nc.sync.dma_start(hbias[:], hbias_p[:])

            # ---- phase A: hT = w1 @ x.T (transposed activations) ----
            with tc.tile_pool(name="pa", bufs=4, space=PSUM) as pa:
                for w1t, kk, ht, ht_8 in (
                    (w1t0, K0, ht0, ht0_8),
                    (w1t1, K1, ht1, ht1_8),
                ):
                    for m in range(kk):
                        pt = pa.tile([128, TOK_PER_CORE], dt.float32, tag="pa")
                        for k in range(K0):
                            nc.tensor.matmul(
                                pt[:],
                                lhsT=w1t[:, k, m * 128:(m + 1) * 128],
                                rhs=xt[:, k, :],
                                start=(k == 0),
                                stop=(k == K0 - 1),
                            )
                        nc.vector.tensor_copy(ht[:, m, :], pt[:])
                        nc.vector.tensor_copy(ht_8[:, m, :], pt[:])

            # ---- phase B: label-logit dots via ones-matmul ----
            lhs_chunks = (
                [xt[:, k, :] for k in range(KX)]
                + [ht0[:, k, :] for k in range(K0)]
                + [ht1[:, k, :] for k in range(K1)]
            )
            with tc.tile_pool(name="pb", bufs=1, space=PSUM) as pb:
                pll = pb.tile([1, TOK_PER_CORE], dt.float32, tag="pb")
                n = len(lhs_chunks)
                for i, lhs in enumerate(lhs_chunks):
                    gt = gs.tile([128, TOK_PER_CORE], dt.bfloat16, tag="g")
                    nc.sync.dma_start(gt[:], gall_p[i])
                    pr = prs.tile([128, TOK_PER_CORE], dt.bfloat16, tag="pr")
                    nc.vector.tensor_tensor(pr[:], lhs, gt[:], op=MULT)
                    nc.tensor.matmul(
                        pll[:], lhsT=ones[:], rhs=pr[:],
                        start=(i == 0), stop=(i == n - 1),
                    )
                nc.vector.tensor_copy(ll[:], pll[:])

            # ---- phase C: streamed fp8-DR logits + fused exp/row-sum ----
            def logit_group(pc, b, kk, lhs3, rhs3, col, bias_rhs=None):
                """64x-scaled logits [128,SUP] via fp8 DoubleRow, exp+rowsum."""
                for sub in range(SUP // 512):
                    sl = slice(sub * 512, (sub + 1) * 512)
                    for c in range(kk // 2):
                        nc.tensor.matmul(
                            pc[:, sl],
                            lhsT=lhs3[:, 2 * c:2 * c + 2, b * 128:(b + 1) * 128],
                            rhs=rhs3[:, 2 * c:2 * c + 2, sl],
                            start=(c == 0),
                            stop=(c == kk // 2 - 1 and bias_rhs is None),
                            perf_mode=DR,
                        )
                    if bias_rhs is not None:
                        # ones-row of xt chunk 8 x (64*head_b) row, bf16
                        nc.tensor.matmul(
                            pc[:, sl],
                            lhsT=xt[:, K0, b * 128:(b + 1) * 128],
                            rhs=bias_rhs[:, sl],
                            start=False, stop=True,
                        )
                nc.scalar.activation(
                    pc[:], pc[:], EXP, scale=1.0 / WSCALE,
                    accum_out=sall[:, col:col + 1],
                )

            with tc.tile_pool(name="pc", bufs=2, space=PSUM) as pcp:
                # head: resident fp8 weights + bf16 bias matmul
                for b in range(N_BLK):
                    pc = pcp.tile([128, SUP], dt.float32, tag="pc")
                    logit_group(pc, b, K0, xt8, hwt8, b * COLS_PER_BLK, bias_rhs=hbias)

                # tail0
                for sup in range(N_SUP0):
                    wt = w2s0.tile([128, K0, SUP], dt.float8e4, tag="w0")
                    for k in range(K0):
                        nc.sync.dma_start(wt[:, k, :], w2t0_p[k, :, sup * SUP:(sup + 1) * SUP])
                    for b in range(N_BLK):
                        pc = pcp.tile([128, SUP], dt.float32, tag="pc")
                        logit_group(pc, b, K0, ht0_8, wt, b * COLS_PER_BLK + 1 + sup)

                # tail1
                for sup in range(N_SUP1):
                    wt = w2s1.tile([128, K1, SUP], dt.float8e4, tag="w1")
                    for k in range(K1):
                        nc.sync.dma_start(wt[:, k, :], w2t1_p[k, :, sup * SUP:(sup + 1) * SUP])
                    for b in range(N_BLK):
                        pc = pcp.tile([128, SUP], dt.float32, tag="pc")
                        logit_group(pc, b, K1, ht1_8, wt, b * COLS_PER_BLK + 1 + N_SUP0 + sup)

            nc.sync.dma_start(out_s_p[:], sall[:])
            nc.sync.dma_start(out_ll_p[:], ll[:])

    nc.compile()
    return nc


def _prep_inputs(w_in, target, head_w, head_b, tail0_w1, tail0_w2, tail1_w1, tail1_w2):
    """Host-side shard + transpose + pad + cast. Returns in_maps + masks."""
    f32 = np.float32
    w_in = np.asarray(w_in, f32)
    target = np.asarray(target).astype(np.int64)
    head_w = np.asarray(head_w, f32)
    head_b = np.asarray(head_b, f32)
    t0w1 = np.asarray(tail0_w1, f32)
    t0w2 = np.asarray(tail0_w2, f32)
    t1w1 = np.asarray(tail1_w1, f32)
    t1w2 = np.asarray(tail1_w2, f32)

    c0, c1, c2 = CUTOFF
    mask0 = (target >= c0) & (target < c1)
    mask1 = (target >= c1) & (target < c2)
    label0 = np.clip(target - c0, 0, c1 - c0 - 1)
    label1 = np.clip(target - c1, 0, c2 - c1 - 1)
    first_t = np.where(mask0, c0, np.where(mask1, c0 + 1, target))

    # label-gathered rows, masks folded in
    g0 = t0w2[label0] * mask0[:, None].astype(f32)     # [N_TOK, 1024]
    g1 = t1w2[label1] * mask1[:, None].astype(f32)     # [N_TOK, 256]
    gh = head_w[first_t]                               # [N_TOK, 1024]
    bh = head_b[first_t]                               # [N_TOK]

    def chunks(a, k, dtype=BF16):  # [K*128, F] -> [K, 128, F]
        return np.ascontiguousarray(a.reshape(k, 128, a.shape[1])).astype(dtype)

    w1t0 = chunks(t0w1.T, K0)                          # [8,128,1024] bf16
    w1t1 = chunks(t1w1.T, K0)                          # [8,128,256] bf16
    w2t0 = np.zeros((1024, T0_PAD), f32)
    w2t0[:, :c1 - c0] = t0w2.T * WSCALE
    w2t0 = chunks(w2t0, K0, FP8)                       # [8,128,8192] fp8
    w2t1 = np.zeros((256, T1_PAD), f32)
    w2t1[:, :c2 - c1] = t1w2.T * WSCALE
    w2t1 = chunks(w2t1, K1, FP8)                       # [2,128,40960] fp8
    hwt8 = np.zeros((D, HEAD_PAD), f32)
    n_head = head_w.shape[0]
    hwt8[:, :n_head] = head_w.T * WSCALE
    hwt8 = chunks(hwt8, K0, FP8)                       # [8,128,2048] fp8
    hbias = np.zeros((128, HEAD_PAD), f32)
    hbias[0, :n_head] = head_b * WSCALE                # pairs with ones-row of xt
    hbias = hbias.astype(BF16)

    in_maps = []
    for c in range(N_CORES):
        sl = slice(c * TOK_PER_CORE, (c + 1) * TOK_PER_CORE)
        xt = np.zeros((KX * 128, TOK_PER_CORE), f32)
        xt[:D] = w_in[sl].T
        xt[D] = 1.0                                    # augmented ones-row (bias)
        ght = np.zeros((KX * 128, TOK_PER_CORE), f32)
        ght[:D] = gh[sl].T
        ght[D] = bh[sl]
        gall = np.concatenate(
            [chunks(ght, KX), chunks(g0[sl].T, K0), chunks(g1[sl].T, K1)], axis=0
        )
        in_maps.append({
            "xt": chunks(xt, KX),
            "xt8": chunks(xt[:D], K0, FP8),
            "w1t0": w1t0, "w1t1": w1t1, "w2t0": w2t0, "w2t1": w2t1,
            "hwt8": hwt8, "hbias": hbias,
            "gall": gall,
        })
    return in_maps, mask0, mask1


def _combine(results, mask0, mask1):
    """Host-side unshard: log in f64, apply masks, mean."""
    total = 0.0
    pad_h = HEAD_PAD - (CUTOFF[0] + 2)
    pad_0 = T0_PAD - (CUTOFF[1] - CUTOFF[0])
    pad_1 = T1_PAD - (CUTOFF[2] - CUTOFF[1])
    for c in range(N_CORES):
        S = results[c]["out_s"].astype(np.float64).reshape(128, N_BLK, COLS_PER_BLK)
        llv = results[c]["out_ll"].astype(np.float64).reshape(N_BLK, 128)
        Sh = S[:, :, 0] - pad_h                        # [128, N_BLK]
        S0 = S[:, :, 1:1 + N_SUP0].sum(-1) - pad_0
        S1 = S[:, :, 1 + N_SUP0:].sum(-1) - pad_1
        # token (p, b) -> global index c*512 + b*128 + p
        idx = (c * TOK_PER_CORE + np.arange(N_BLK)[None, :] * 128
               + np.arange(128)[:, None])
        m0 = mask0[idx]
        m1 = mask1[idx]
        nll = np.log(Sh) + m0 * np.log(S0) + m1 * np.log(S1) - llv.T
        total += nll.sum()
    return np.float32(total / N_TOK)


def _run(inputs, trace=False):
    from concourse.bass_utils import run_bass_kernel_spmd

    if "nc" not in _cache:
        _cache["nc"] = _build_nc()
    nc = _cache["nc"]
    in_maps, mask0, mask1 = _prep_inputs(**inputs)
    res = run_bass_kernel_spmd(nc, in_maps, core_ids=list(range(N_CORES)), trace=trace)
    loss = _combine(res.results, mask0, mask1)
    return loss, res


def kernel(**inputs) -> np.ndarray:
    loss, _ = _run(inputs, trace=False)
    return loss
